# revision 37
# baseline (speedup 1.0000x reference)
"""Trainium2 Bass kernel for nn_Attention_11991548690893.

Reference semantics (faithfully-reproduced bug): q = k = v = the key
projection, so only the middle third of W_attn is used and the attention
matrix S = kh @ kh^T is SYMMETRIC.  Design:
  - Megatron head-sharding: core c owns heads 2c, 2c+1 (128 head-dims),
    computes a partial output against its 128 rows of W_proj; the host
    sums the 8 bf16 partials and adds b_proj.
  - bf16 matmul operands everywhere (PSUM accumulation stays fp32);
    rel-err gate is 2e-2, bf16 lands ~4e-3.
  - Symmetry: only the upper-triangular blocks of S are matmul'd and
    exp'd (136 of 256 per batch-head). The mirrored lower blocks of
    exp(S) are produced by xbar DMA transposes (SBUF->SBUF, off the
    PE/ACT critical engines). This rebalances ACT (exp) from ~128us to
    ~77us so the PE stream stays dense and the HAM clock stays warm.
  - k natural-layout blocks (outT stationary) built by DMA transpose
    of kT; the ones column accumulates softmax denominators in PSUM
    partition 64 during the second attention matmul.
  - Normalization: single-partition denom row -> fast-approx
    reciprocal -> gpsimd partition_broadcast -> one DVE multiply.
    No DRAM bounce, no slow iterative divide.
"""

import numpy as np
import ml_dtypes

import concourse.bass as bass
import concourse.mybir as mybir
import concourse.tile as tile
from concourse import bacc
from concourse.bass_utils import run_bass_kernel_spmd

F32 = mybir.dt.float32
BF16 = mybir.dt.bfloat16
EXP = mybir.ActivationFunctionType.Exp

B = 2
L = 2048
D = 1024
H = 16
DH = 64
NCORES = 8
DHC = 128            # head-dims per core (2 heads x 64)
L2 = B * L           # 4096
P = 128
NBLK = L // P        # 16 l-blocks per batch
SCALE = 1.0 / np.sqrt(DH)   # 0.125


def _build_kernel(ctx, tc, xT, wk, bk, wp, out):
    nc = tc.nc

    singles = ctx.enter_context(tc.tile_pool(name="singles", bufs=1))
    xpool = ctx.enter_context(tc.tile_pool(name="xpool", bufs=2))
    otpool = ctx.enter_context(tc.tile_pool(name="otpool", bufs=6))
    otfpool = ctx.enter_context(tc.tile_pool(name="otfpool", bufs=3))
    npool = ctx.enter_context(tc.tile_pool(name="npool", bufs=1))
    scratch = ctx.enter_context(tc.tile_pool(name="scratch", bufs=2))
    ps_mm = ctx.enter_context(tc.tile_pool(name="ps_mm", bufs=3, space="PSUM"))
    ps_ot = ctx.enter_context(tc.tile_pool(name="ps_ot", bufs=1, space="PSUM"))

    wk_sb = singles.tile([P, 8, P], BF16)     # W_k slice, D-major tiles
    nc.scalar.dma_start(wk_sb, wk.rearrange("(o p) m -> p o m", p=P))
    bk_sb = singles.tile([P, 1], F32)
    nc.scalar.dma_start(bk_sb, bk)
    wp_sb = singles.tile([P, D], BF16)        # W_proj rows, all 128 head-dims
    nc.scalar.dma_start(wp_sb, wp)

    kT = singles.tile([P, B, L], BF16)        # [128 dh, batch, tok]
    # zero-padded per-head copies of kT: K=128 stationaries keep the full
    # PE array active (K=64 stationaries leave HAM permanently throttled)
    kTz = singles.tile([P, 2, B, L], BF16)    # [128, h2, batch, tok]
    nc.vector.memset(kTz.rearrange("p h b l -> p (h b l)"), 0.0)
    # knat: [tok, blk(b*16+m), h2, {64 kh cols | ones | pad}]
    knat = singles.tile([P, 2 * NBLK, 2, 66], BF16)
    # whole-tile memset to 1.0 (contiguous): col 64 becomes the ones column
    # for the denominator row; cols 0:64 are overwritten by the transposes.
    nc.vector.memset(knat.rearrange("p a b c -> p (a b c)"), 1.0)
    strips = singles.tile([P, NBLK, L], BF16)  # exp(S) k-strips, q-free
    osb = singles.tile([P, NBLK // 2, D], BF16)  # phase-3 staging (half batch)

    # ---- Phase 1 (per batch): kT = (x @ Wk + bk)^T, then kTz + knat ----
    xTr = xT.rearrange("(o p) l -> p o l", p=P)   # [128, 8, 4096]

    def ph1_batch(b_):
        aps = None
        for lc2 in range(4):
            lc = b_ * 4 + lc2
            xc = xpool.tile([P, 8, 512], BF16, tag="xc")
            # batch 0 alternates DMA rings (both idle at startup); batch 1
            # stays on the ACT ring so it never queues behind mirrors
            eng = nc.sync if (b_ == 0 and lc2 % 2 == 1) else nc.scalar
            eng.dma_start(xc, xTr[:, :, lc * 512:(lc + 1) * 512])
            if lc2 % 2 == 0:
                aps = ps_mm.tile([P, 1024], F32, tag="mm")
            for dc in range(8):
                nc.tensor.matmul(
                    aps[:, (lc2 % 2) * 512:(lc2 % 2 + 1) * 512],
                    wk_sb[:, dc],
                    xc[:, dc],
                    start=(dc == 0),
                    stop=(dc == 7),
                )
            if lc2 % 2 == 1:
                hl = lc2 // 2
                nc.vector.tensor_scalar_add(
                    kT[:, b_, hl * 1024:hl * 1024 + 1024], aps, bk_sb)
                for h2 in range(2):
                    nc.vector.tensor_copy(
                        kTz[h2 * DH:(h2 + 1) * DH, h2, b_,
                            hl * 1024:(hl + 1) * 1024],
                        kT[h2 * DH:(h2 + 1) * DH, b_,
                           hl * 1024:(hl + 1) * 1024])
        # knat via DMA transpose of kT (chunk-major rows land as
        # [tok%128, blk]). The xbar ignores sub-4KB mid-dim strides on
        # the destination, so transpose into a contiguous staging tile
        # and DVE-copy into the 66-wide assembled layout.
        for h2 in range(2):
            knd = scratch.tile([P, NBLK, 64], BF16, tag="knd")
            nc.sync.dma_start_transpose(
                knd,
                kT[h2 * DH:(h2 + 1) * DH, b_, :],
            )
            nc.vector.tensor_copy(
                knat[:, b_ * NBLK:(b_ + 1) * NBLK, h2, 0:64], knd)

    # ---- Phase 2: attention per (batch, head-pair); S upper-tri only ----
    oth_tiles = {}

    def do_bh(b_, h2, ph3_after=None, ph3_final=None):
        khT = kT[:, b_, :]                        # [128, 2048] both heads
        statz = kTz[:, h2, b_, :]                 # [128, 2048] head h2 + zeros

        def att(m):
            off = m * P
            span = L - off
            stat = statz[:, off:off + P]
            for c0 in range(0, span, 1024):
                cw = min(1024, span - c0)
                aps = ps_mm.tile([P, 1024], F32, tag="mm")
                for n0 in range(0, cw, 512):
                    w = min(512, cw - n0)
                    nc.tensor.matmul(
                        aps[:, n0:n0 + w],
                        stat,
                        khT[:, off + c0 + n0:off + c0 + n0 + w],
                        start=True,
                        stop=True,
                    )
                nc.scalar.activation(
                    strips[:, m, off + c0:off + c0 + cw], aps[:, 0:cw],
                    EXP, scale=SCALE)
            if m < NBLK - 1:
                # mirror exp'd blocks (m, n>m) into later strips via xbar
                nc.sync.dma_start_transpose(
                    strips[:, m + 1:NBLK, off:off + P],
                    strips[:, m, off + P:L],
                )

        def outT(ot, half, m):
            lhsT = knat[:, b_ * NBLK + m, h2, 0:65]
            for n0 in (0, 512):
                nc.tensor.matmul(
                    ot[:, n0:n0 + 512],
                    lhsT,
                    strips[:, m, half * 1024 + n0:half * 1024 + n0 + 512],
                    start=(m == 0),
                    stop=(m == NBLK - 1),
                    skip_group_check=True,
                )

        def normalize(ot, half):
            # row copy first so the reciprocal chain starts immediately; the
            # full copy then frees the PSUM slot for the next accumulation
            nrow = npool.tile([1, 1024], F32, tag="nrow")
            nc.vector.tensor_copy(nrow, ot[DH:DH + 1, :])
            otc = npool.tile([DH, 1024], F32, tag="otc")
            nc.vector.tensor_copy(otc, ot[0:DH, :])
            rec = npool.tile([1, 1024], F32, tag="rec")
            nc.vector.reciprocal_approx_fast(rec, nrow)
            bc = npool.tile([DH, 1024], F32, tag="bc")
            nc.gpsimd.partition_broadcast(bc, rec)
            oth = otpool.tile([DH, 1024], BF16, tag="oth")
            nc.vector.tensor_mul(oth, otc, bc)
            oth_tiles[(b_, h2, half)] = oth

        # q-half 0 accumulates inside the m-loop (outT lags att by LAG so
        # the exp + mirror pipeline never stalls the PE); q-half 1 runs as
        # a dense post-loop PE pass over the persisted strips.
        LAG = 6
        ot0 = ps_ot.tile([DH + 1, 1024], F32, tag="ot")
        for m in range(LAG):
            att(m)
        for m in range(LAG, NBLK):
            outT(ot0, 0, m - LAG)
            att(m)
        for m in range(NBLK - LAG, NBLK):
            outT(ot0, 0, m)
        normalize(ot0, 0)
        ot1 = ps_ot.tile([DH + 1, 1024], F32, tag="ot")
        for m in range(NBLK):
            outT(ot1, 1, m)
        if ph3_final is not None:
            ph3(ph3_final, (0,))
        normalize(ot1, 1)
        if ph3_after is not None:
            ph3(ph3_after, (0, 1))
        if ph3_final is not None:
            ph3(ph3_final, (1,))

    # ---- Phase 3: partial = [oth_h0; oth_h1]^T.T @ Wp (K=128) per batch ----
    def ph3(b_, halves=(0, 1)):
        for half in halves:
            othf = otfpool.tile([P, 1024], BF16, tag="othf")
            nc.vector.tensor_copy(othf[0:DH, :], oth_tiles[(b_, 0, half)])
            nc.vector.tensor_copy(othf[DH:P, :], oth_tiles[(b_, 1, half)])
            for q8 in range(NBLK // 2):
                pps = ps_mm.tile([P, 1024], F32, tag="mm")
                for n2 in range(2):
                    nc.tensor.matmul(
                        pps[:, n2 * 512:(n2 + 1) * 512],
                        othf[:, q8 * P:q8 * P + P],
                        wp_sb[:, n2 * 512:(n2 + 1) * 512],
                        start=True,
                        stop=True,
                    )
                if q8 % 2 == 0:
                    nc.vector.tensor_copy(osb[:, q8], pps)
                else:
                    nc.scalar.activation(
                        osb[:, q8], pps, mybir.ActivationFunctionType.Copy)
                if q8 % 4 == 3:
                    r0 = b_ * L + half * (L // 2) + (q8 - 3) * P
                    nc.scalar.dma_start(
                        out[r0:r0 + 4 * P, :].rearrange("(q p) d -> p q d", p=P),
                        osb[:, q8 - 3:q8 + 1],
                    )

    # Emission order: batch-1's phase 1 fills the first bh boundary; ph3(0)
    # is emitted after bh(1,0) so its dense matmuls land in the batch
    # boundary window instead of stalling on the normalize chain.
    ph1_batch(0)
    do_bh(0, 0)
    ph1_batch(1)
    do_bh(0, 1)
    do_bh(1, 0, ph3_after=0)
    do_bh(1, 1, ph3_final=1)


_NC_CACHE = None


def _get_nc():
    global _NC_CACHE
    if _NC_CACHE is None:
        nc = bacc.Bacc("TRN2", target_bir_lowering=False)
        xT = nc.dram_tensor("xt", [D, L2], BF16, kind="ExternalInput").ap()
        wk = nc.dram_tensor("wk", [D, DHC], BF16, kind="ExternalInput").ap()
        bk = nc.dram_tensor("bk", [DHC, 1], F32, kind="ExternalInput").ap()
        wp = nc.dram_tensor("wp", [DHC, D], BF16, kind="ExternalInput").ap()
        out = nc.dram_tensor("out", [L2, D], BF16, kind="ExternalOutput").ap()
        from contextlib import ExitStack
        with tile.TileContext(nc) as tc, ExitStack() as ctx:
            _build_kernel(ctx, tc, xT, wk, bk, wp, out)
        nc.compile()
        _NC_CACHE = nc
    return _NC_CACHE


def _run(inputs, trace=False):
    x = np.asarray(inputs["x"], dtype=np.float32)
    W_attn = np.asarray(inputs["W_attn"], dtype=np.float32)
    b_attn = np.asarray(inputs["b_attn"], dtype=np.float32)
    W_proj = np.asarray(inputs["W_proj"], dtype=np.float32)
    b_proj = np.asarray(inputs["b_proj"], dtype=np.float32)

    bf16 = ml_dtypes.bfloat16
    xT = x.reshape(L2, D).T.astype(bf16)                     # [1024, 4096]
    Wk = W_attn[:, D:2 * D]                                  # [1024, 1024]
    bk = b_attn[D:2 * D]                                     # [1024]

    in_maps = []
    for c in range(NCORES):
        sl = slice(c * DHC, (c + 1) * DHC)
        in_maps.append({
            "xt": xT,
            "wk": Wk[:, sl].astype(bf16),
            "bk": np.ascontiguousarray(bk[sl]).reshape(DHC, 1),
            "wp": W_proj[sl, :].astype(bf16),
        })

    nc = _get_nc()
    res = run_bass_kernel_spmd(nc, in_maps, core_ids=list(range(NCORES)),
                               trace=trace)
    acc = np.zeros((L2, D), dtype=np.float32)
    for r in res.results:
        acc += r["out"].astype(np.float32)
    acc += b_proj
    return acc.reshape(B, L, D), res


def kernel(**inputs):
    out, _ = _run(inputs, trace=False)
    return out


def kernel_traced(**inputs):
    return _run(inputs, trace=True)
